# revision 71
# baseline (speedup 1.0000x reference)
"""LongFormer sliding-window attention on 8 Trainium2 NeuronCores.

Sharding: batch*heads data-parallel. 24 (batch, head) pairs -> 8 cores,
each core owns one batch (core//4) and 3 consecutive heads (3*(core%4)).
No collectives.

Per-core kernel (v5):
  - All matmuls run in fp16 (1 PE cycle/row at any moving width, vs
    fp32r's 4x penalty below 256 columns); accumulation in f32 PSUM.
    (fp8 DoubleRow projection worked in CoreSim but is rejected by this
    environment's NEFF executor, so it is disabled.)
  - x arrives host-transposed as fp16 [768, 4096]; Q,K land in
    transposed SBUF layout qkT (3 groups of 128 partitions: [q_h0|q_h1],
    [k_h0|k_h1], [q_h2|k_h2]; k_h2 is DMA-shifted to partitions 0:64 so
    attention matmul operands share a base partition).  V is projected
    token-major [tok, 3*65]; the softmax-denominator ones column and bv
    are added by the PSUM->SBUF copy.
  - Scores for a 256-query chunk are computed TRANSPOSED [kpos, q],
    t-major, into a 2-bank PSUM tile ([t1|t2|t3|t4], 256 queries each)
    so the interior softmax exp is a single ~1k-col Activation op; the
    two 128-col diagonal corner tiles for heads 0+1 share one PSUM bank
    and one exp; triangle masks are fp16 multiplies on DVE; PV takes E
    slices as stationary operands.
  - Emission interleaves projection pieces with per-head attention so
    the PE always has dependency-free work while Act chews softmax;
    attention lags projection by one stripe; odd rounds carry a third
    chunk; chunk 0 is held back for the tail.
"""

import os
import sys

import numpy as np
import ml_dtypes

sys.path.insert(0, "/opt/trn_rl_repo")

import concourse.bass as bass  # noqa: E402
import concourse.tile as tile  # noqa: E402
from concourse import bacc, mybir  # noqa: E402
from concourse import bass_utils  # noqa: E402

B, S, E = 2, 4096, 768
H, D = 12, 64
W2 = 256            # one-sided window w
C = S // W2         # 16 chunks of 256 queries
HPC = 3             # heads per core
N_CORES = 8

f32 = mybir.dt.float32
f16 = mybir.dt.float16
f8 = mybir.dt.float8e4

KT = 6              # 768 = 6 k-tiles of 128
NT = 8              # 4096 = 8 stripes of 512 tokens
VW = 65 * HPC       # packed v width: 3 heads x (64 dims + ones col) = 195
VOFF = (0, 195, 512, 707)   # V row-tile offsets in the PSUM tile (bank-safe)
AEXP = mybir.ActivationFunctionType.Exp
ADD = mybir.AluOpType.add
MUL = mybir.AluOpType.mult
DROW = mybir.MatmulPerfMode.DoubleRow

USE_FP8_QKPROJ = False
USE_DROW = False
DROW_GROUPS = ()
V3_PV = True        # bisect flag: per-qh recip/scale + two f32 out DMAs


def _build_body(tc, aps):
    nc = tc.nc
    (xt_d, xt8_d, wqk_d, wqk8_d, bqk_d, wv_d, bvt_d, masks_d, out_d) = aps

    from contextlib import ExitStack
    ctx = ExitStack()
    sb = ctx.enter_context(tc.tile_pool(name="sb", bufs=1))
    xnat_p = ctx.enter_context(tc.tile_pool(name="xnat", bufs=3))
    e_p = ctx.enter_context(tc.tile_pool(name="ep", bufs=6))
    ec_p = ctx.enter_context(tc.tile_pool(name="ecp", bufs=3))
    out_p = ctx.enter_context(tc.tile_pool(name="outp", bufs=4))
    ps_s = ctx.enter_context(tc.tile_pool(name="pss", bufs=3, space="PSUM"))
    ps_cp = ctx.enter_context(tc.tile_pool(name="pscp", bufs=2, space="PSUM"))

    # ---- persistent SBUF tensors ----
    if USE_FP8_QKPROJ:
        wqk = sb.tile([128, KT * 384], f8, tag="wqk")
    else:
        wqk = sb.tile([128, KT * 384], f16, tag="wqk")
    qkT = sb.tile([128, 3 * S], f16, tag="qkT")
    kh2 = sb.tile([128, S], f16, tag="kh2")     # k_h2 shifted to parts 0:64
    vsb = sb.tile([128, 2 * C * VW], f16, tag="vsb")    # 32 row-tiles
    mask = sb.tile([128, 256], f16, tag="mask")    # [tril | triu]
    mask2 = sb.tile([128, 512], f16, tag="mask2")  # [tril|triu|tril|triu]
    wv = sb.tile([128, KT * VW], f16, tag="wv")
    bvt = sb.tile([128, 2 * VW], f16, tag="bvt")  # [bv|1]x3 heads, x2 rts
    bqk = sb.tile([128, 3], f32, tag="bqk")

    def dma_stripe8(m):
        if not USE_FP8_QKPROJ:
            return None
        x8 = xnat_p.tile([128, KT * 512], f8, tag="x8", name="x8")
        nc.sync.dma_start(
            x8[:].rearrange("p (k c) -> p k c", k=KT),
            xt8_d[:, m * 512:(m + 1) * 512].rearrange(
                "(k p) c -> p k c", p=128))
        return x8

    def dma_stripe16(m, nsplit=1):
        xTn = xnat_p.tile([128, KT * 512], f16, tag="xTn", name="xTn")
        kk = KT // nsplit
        for k2 in range(nsplit):
            nc.sync.dma_start(
                xTn[:, k2 * kk * 512:(k2 + 1) * kk * 512].rearrange(
                    "p (k c) -> p k c", k=kk),
                xt_d[k2 * kk * 128:(k2 + 1) * kk * 128,
                     m * 512:(m + 1) * 512].rearrange(
                    "(k p) c -> p k c", p=128))
        return xTn

    def dma_stripe(m):
        x8 = dma_stripe8(m)
        xTn = dma_stripe16(m)
        return xTn, (x8 if USE_FP8_QKPROJ else xTn)

    def dma_wqk(half):
        src = wqk8_d if USE_FP8_QKPROJ else wqk_d
        nc.sync.dma_start(
            wqk[:, half * 3 * 384:(half + 1) * 3 * 384].rearrange(
                "p (k c) -> p k c", k=3),
            src[half * 384:(half + 1) * 384, :].rearrange(
                "(k p) c -> p k c", p=128))

    def dma_consts():
        nc.sync.dma_start(bqk[:], bqk_d[:].rearrange("g p -> p g"))
        nc.sync.dma_start(mask[:], masks_d[:, 0:256])
        nc.sync.dma_start(mask2[:], masks_d[:, 256:768])
        nc.sync.dma_start(
            wv[:].rearrange("p (k c) -> p k c", k=KT),
            wv_d[:].rearrange("(k p) c -> p k c", p=128))
        nc.sync.dma_start(bvt[:], bvt_d[:])

    def q_slice(hi, lo, n):
        if hi < 2:
            return qkT[64 * hi:64 * hi + 64, lo:lo + n]
        return qkT[0:64, 2 * S + lo:2 * S + lo + n]

    def k_slice(hi, lo, n):
        if hi < 2:
            return qkT[64 * hi:64 * hi + 64, S + lo:S + lo + n]
        return kh2[0:64, lo:lo + n]

    def proj_qk(m, x8, sg, g, off):
        if USE_FP8_QKPROJ and (USE_DROW or g in DROW_GROUPS):
            wv8 = wqk[:].rearrange("p (k c) -> p k c", k=KT)
            xv8 = x8[:].rearrange("p (k c) -> p k c", k=KT)
            for kp in range(KT // 2):
                nc.tensor.matmul(
                    sg[:, off:off + 512],
                    wv8[:, 2 * kp:2 * kp + 2, g * 128:(g + 1) * 128],
                    xv8[:, 2 * kp:2 * kp + 2, :],
                    start=(kp == 0), stop=(kp == KT // 2 - 1),
                    perf_mode=DROW,
                )
        else:
            for kt in range(KT):
                nc.tensor.matmul(
                    sg[:, off:off + 512],
                    wqk[:, kt * 384 + g * 128: kt * 384 + g * 128 + 128],
                    x8[:, kt * 512:(kt + 1) * 512],
                    start=(kt == 0), stop=(kt == KT - 1),
                )

    def proj_qk_copy(m, sg, g, off):
        nc.vector.tensor_scalar_add(
            qkT[:, g * S + m * 512: g * S + m * 512 + 512],
            sg[:, off:off + 512], bqk[:, g:g + 1])
        if g == 2:
            # k_h2 lives at parts 64:128 of G2; matmul operands must
            # share a base partition: shift to parts 0:64 via SBUF DMA.
            nc.sync.dma_start(
                kh2[0:64, m * 512:(m + 1) * 512],
                qkT[64:128, 2 * S + m * 512: 2 * S + (m + 1) * 512])

    def proj_qk_ab(m, x8):
        sa = ps_s.tile([128, 1024], f32, tag="S", name="sa")
        proj_qk(m, x8, sa, 0, 0)
        proj_qk(m, x8, sa, 1, 512)
        proj_qk_copy(m, sa, 0, 0)
        proj_qk_copy(m, sa, 1, 512)

    def proj_qk_c(m, x8):
        sq = ps_s.tile([128, 1024], f32, tag="S", name="sq")
        proj_qk(m, x8, sq, 2, 0)
        proj_qk_copy(m, sq, 2, 0)

    def proj_v(m, xTn):
        # 4 V row-tiles of 128 tokens in one 2-bank PSUM tile
        vt = ps_s.tile([128, 1024], f32, tag="S", name="vt")
        for rl in range(4):
            for kt in range(KT):
                nc.tensor.matmul(
                    vt[:, VOFF[rl]:VOFF[rl] + VW],
                    xTn[:, kt * 512 + rl * 128: kt * 512 + rl * 128 + 128],
                    wv[:, kt * VW:(kt + 1) * VW],
                    start=(kt == 0), stop=(kt == KT - 1),
                )
        rt = m * 4
        # adds bv and writes the constant 1.0 denominator columns
        nc.vector.tensor_tensor(
            vsb[:, rt * VW: rt * VW + 2 * VW], vt[:, 0:390], bvt[:], ADD)
        nc.vector.tensor_tensor(
            vsb[:, (rt + 2) * VW: (rt + 4) * VW], vt[:, 512:902], bvt[:], ADD)

    def attn_qk_hi(c, hi, cst):
        # Interior scores, t-major [t1|t2|t3|t4] x 256 queries in a
        # 2-bank tile; corners for hi 0/1 share bt01 ([t0|t5|t0|t5]),
        # hi 2 uses bt2; exp for bt01 runs once, after hi 1.
        tmin = 2 if c == 0 else 1
        tmax = 3 if c == C - 1 else 4
        st = ps_s.tile([128, 1024], f32, tag="S", name="st")
        for t in range(tmin, tmax + 1):
            nc.tensor.matmul(
                st[:, (t - 1) * 256:t * 256],
                k_slice(hi, (2 * (c - 1) + t) * 128, 128),
                q_slice(hi, c * 256, 256),
                start=True, stop=True)
        if hi == 0:
            cst["bt01"] = ps_cp.tile([128, 512], f32, tag="cp", name="bt01")
            cst["ec01"] = ec_p.tile([128, 512], f16, tag="ec01", name="ec01")
        cb = cst["bt01"] if hi < 2 else \
            ps_cp.tile([128, 256], f32, tag="cp", name="bt2")
        co = (hi % 2) * 256
        if c > 0:
            nc.tensor.matmul(
                cb[:, co:co + 128],
                k_slice(hi, (2 * c - 2) * 128, 128),
                q_slice(hi, c * 256, 128),
                start=True, stop=True)
        if c < C - 1:
            nc.tensor.matmul(
                cb[:, co + 128:co + 256],
                k_slice(hi, (2 * c + 3) * 128, 128),
                q_slice(hi, c * 256 + 128, 128),
                start=True, stop=True)
        ei = e_p.tile([128, 1024], f16, tag="ei", name="ei")
        # interior exp: one op over the contiguous valid region
        # (scale folds the 1/sqrt(D) q-scaling into the softmax)
        alo, ahi = (tmin - 1) * 256, tmax * 256
        nc.scalar.activation(ei[:, alo:ahi], st[:, alo:ahi],
                             AEXP, scale=0.125)
        # interior triangle masks: t1 queries 128:256 lower-tri, t4
        # queries 0:128 upper-tri
        if c > 0:
            nc.vector.tensor_mul(ei[:, 128:256], ei[:, 128:256],
                                 mask[:, 0:128])
        if c < C - 1:
            nc.vector.tensor_mul(ei[:, 768:896], ei[:, 768:896],
                                 mask[:, 128:256])
        # corner exp + masks ([t0 lower | t5 upper] pattern)
        def corner_exp(dst, src, width):
            if 0 < c < C - 1:
                nc.scalar.activation(dst[:, 0:width], src[:, 0:width],
                                     AEXP, scale=0.125)
                nc.vector.tensor_mul(dst[:, 0:width], dst[:, 0:width],
                                     mask2[:, 0:width])
            elif c == 0:
                for lo in range(128, width, 256):
                    nc.scalar.activation(dst[:, lo:lo + 128],
                                         src[:, lo:lo + 128],
                                         AEXP, scale=0.125)
                    nc.vector.tensor_mul(dst[:, lo:lo + 128],
                                         dst[:, lo:lo + 128],
                                         mask[:, 128:256])
            else:
                for lo in range(0, width, 256):
                    nc.scalar.activation(dst[:, lo:lo + 128],
                                         src[:, lo:lo + 128],
                                         AEXP, scale=0.125)
                    nc.vector.tensor_mul(dst[:, lo:lo + 128],
                                         dst[:, lo:lo + 128],
                                         mask[:, 0:128])
        if hi == 1:
            corner_exp(cst["ec01"], cst["bt01"], 512)
            ec = cst["ec01"]
        elif hi == 2:
            ec = ec_p.tile([128, 256], f16, tag="ec2", name="ec2")
            corner_exp(ec, cb, 256)
        else:
            ec = cst["ec01"]
        return st, ei, ec

    def attn_pv_hi(c, hi, sthi, ots):
        tmin = 2 if c == 0 else 1
        tmax = 3 if c == C - 1 else 4
        _, ei, ec = sthi
        eco = (hi % 2) * 256 if hi < 2 else 0
        po = ps_cp.tile([128, 130], f32, tag="cp", name="po")
        for qh in range(2):
            esls = []
            if qh == 0 and c > 0:
                esls.append((ec[:, eco:eco + 128], 2 * (c - 1)))
            for t in range(tmin, tmax + 1):
                esls.append((ei[:, (t - 1) * 256 + qh * 128:
                                (t - 1) * 256 + qh * 128 + 128],
                             2 * (c - 1) + t))
            if qh == 1 and c < C - 1:
                esls.append((ec[:, eco + 128:eco + 256], 2 * c + 3))
            for i, (esl, kt_abs) in enumerate(esls):
                nc.tensor.matmul(
                    po[:, qh * 65:qh * 65 + 65],
                    esl,
                    vsb[:, kt_abs * VW + hi * 65:
                        kt_abs * VW + (hi + 1) * 65],
                    start=(i == 0), stop=(i == len(esls) - 1),
                )
        rec = e_p.tile([128, 2], f32, tag="rec", name="rec")
        if V3_PV:
            nc.vector.reciprocal(rec[:, 0:1], po[:, 64:65])
            nc.vector.reciprocal(rec[:, 1:2], po[:, 129:130])
            for qh in range(2):
                nc.vector.tensor_scalar_mul(
                    ots[:, qh * 192 + hi * 64: qh * 192 + (hi + 1) * 64],
                    po[:, qh * 65:qh * 65 + 64], rec[:, qh:qh + 1])
        else:
            po_v = po[:].rearrange("p (q d) -> p q d", d=65)
            rec_v = rec[:].rearrange("p (q o) -> p q o", o=1)
            nc.vector.reciprocal(rec_v, po_v[:, :, 64:65])
            # both query-halves scaled in one op: out[p, qh, d] =
            # po[p, qh, d] * rec[p, qh]
            nc.vector.tensor_tensor(
                ots[:].rearrange("p (q j) -> p q j", q=2)[:, :,
                                                          hi * 64:
                                                          hi * 64 + 64],
                po_v[:, :, 0:64], rec_v.broadcast_to([128, 2, 64]), MUL)

    def new_ots():
        return out_p.tile([128, 384], f16, tag="ot", name="ot")

    def dma_out(c, ots):
        nc.sync.dma_start(
            out_d[c * 256:(c + 1) * 256, :].rearrange(
                "(q p) j -> p q j", p=128),
            ots[:].rearrange("p (q j) -> p q j", q=2))

    def attn_qk_chunk(c):
        cst = {}
        return [attn_qk_hi(c, hi, cst) for hi in range(HPC)]

    def emit_round(m, xTn, x8, vs, c1, c2, ce=None):
        """One steady-state round for stripe m and chunk pair
        (c1, c2) = (2m-2, 2m-1).  c1 depends only on earlier stripes;
        c2's halo (chunk 2m) lands with this stripe's projection, whose
        pieces are interleaved ahead of it.  vs lists the V-projection
        stripes to emit (normally just [m]); ce appends a third chunk."""
        cst1, cst2 = {}, {}
        s1 = [None] * HPC
        s2 = [None] * HPC
        o1, o2 = new_ots(), new_ots()
        s1[0] = attn_qk_hi(c1, 0, cst1)
        proj_qk_ab(m, x8)
        s1[1] = attn_qk_hi(c1, 1, cst1)
        proj_qk_c(m, x8)
        s1[2] = attn_qk_hi(c1, 2, cst1)
        if len(vs) > 1:
            proj_v(*vs[1])
        attn_pv_hi(c1, 0, s1[0], o1)
        s2[0] = attn_qk_hi(c2, 0, cst2)
        proj_v(*vs[0])
        attn_pv_hi(c1, 1, s1[1], o1)
        s2[1] = attn_qk_hi(c2, 1, cst2)
        attn_pv_hi(c1, 2, s1[2], o1)
        s2[2] = attn_qk_hi(c2, 2, cst2)
        dma_out(c1, o1)
        if ce is None:
            attn_pv_hi(c2, 0, s2[0], o2)
            attn_pv_hi(c2, 1, s2[1], o2)
            attn_pv_hi(c2, 2, s2[2], o2)
            dma_out(c2, o2)
            return
        cste = {}
        se = [None] * HPC
        oe = new_ots()
        attn_pv_hi(c2, 0, s2[0], o2)
        se[0] = attn_qk_hi(ce, 0, cste)
        attn_pv_hi(c2, 1, s2[1], o2)
        se[1] = attn_qk_hi(ce, 1, cste)
        attn_pv_hi(c2, 2, s2[2], o2)
        se[2] = attn_qk_hi(ce, 2, cste)
        dma_out(c2, o2)
        attn_pv_hi(ce, 0, se[0], oe)
        attn_pv_hi(ce, 1, se[1], oe)
        attn_pv_hi(ce, 2, se[2], oe)
        dma_out(ce, oe)

    def emit_attn_pair(c1, c2):
        cst1, cst2 = {}, {}
        s1 = [attn_qk_hi(c1, hi, cst1) for hi in range(HPC)]
        o1, o2 = new_ots(), new_ots()
        s2 = [None] * HPC
        attn_pv_hi(c1, 0, s1[0], o1)
        s2[0] = attn_qk_hi(c2, 0, cst2)
        attn_pv_hi(c1, 1, s1[1], o1)
        s2[1] = attn_qk_hi(c2, 1, cst2)
        attn_pv_hi(c1, 2, s1[2], o1)
        s2[2] = attn_qk_hi(c2, 2, cst2)
        dma_out(c1, o1)
        attn_pv_hi(c2, 0, s2[0], o2)
        attn_pv_hi(c2, 1, s2[1], o2)
        attn_pv_hi(c2, 2, s2[2], o2)
        dma_out(c2, o2)

    # ---- pipelined emission ----
    # DMA priority: fp8 x + weights gate the first projections (and the
    # kh2 shift must hit the DMA queue early); masks gate round 1's
    # first exp; wv/bvt gate V(0); fp16 x stripes stream in behind.
    # Attention pair (2m-2, 2m-1) runs in round m >= 1; round 7 carries
    # chunk 14 as well, leaving only chunk 15 for the tail.
    dma_wqk(0)
    if USE_FP8_QKPROJ:
        x80 = dma_stripe8(0)
        xTn0 = dma_stripe16(0, nsplit=3)
    else:
        xTn0 = x80 = dma_stripe16(0, nsplit=3)
    dma_wqk(1)
    dma_consts()
    stripes = [(xTn0, x80), dma_stripe(1)]
    proj_qk_ab(0, x80)
    proj_qk_c(0, x80)
    for m in range(1, NT):
        if m + 1 < NT:
            stripes.append(dma_stripe(m + 1))
        xTn, x8 = stripes[m]
        vs = [(m, xTn)]
        if m == 1:
            vs.append((0, xTn0))
        emit_round(m, xTn, x8, vs, 2 * m - 2, 2 * m - 1)
    emit_attn_pair(2 * NT - 2, 2 * NT - 1)      # (14, 15)
    ctx.close()


def build_program():
    nc = bacc.Bacc("TRN2", target_bir_lowering=False, debug=False)
    xt_d = nc.dram_tensor("xt", [E, S], f16, kind="ExternalInput").ap()
    if USE_FP8_QKPROJ:
        xt8_d = nc.dram_tensor("xt8", [E, S], f8, kind="ExternalInput").ap()
        wqk_d = None
        wqk8_d = nc.dram_tensor("wqk8", [E, 384], f8,
                                kind="ExternalInput").ap()
    else:
        xt8_d = None
        wqk_d = nc.dram_tensor("wqk", [E, 384], f16,
                               kind="ExternalInput").ap()
        wqk8_d = None
    bqk_d = nc.dram_tensor("bqk", [3, 128], f32, kind="ExternalInput").ap()
    wv_d = nc.dram_tensor("wv", [E, VW], f16, kind="ExternalInput").ap()
    bvt_d = nc.dram_tensor("bvt", [128, 2 * VW], f16,
                           kind="ExternalInput").ap()
    masks_d = nc.dram_tensor("masks", [128, 768], f16,
                             kind="ExternalInput").ap()
    out_d = nc.dram_tensor("out", [S, 192], f16, kind="ExternalOutput").ap()
    with tile.TileContext(nc) as tc:
        _build_body(tc, (xt_d, xt8_d, wqk_d, wqk8_d, bqk_d, wv_d, bvt_d,
                         masks_d, out_d))
    nc.compile()
    return nc


def make_in_maps(hidden_states, Wq, bq, Wk, bk, Wv, bv):
    hs = np.asarray(hidden_states, np.float32)
    Wq = np.asarray(Wq, np.float32)
    Wk = np.asarray(Wk, np.float32)
    Wv = np.asarray(Wv, np.float32)
    bq = np.asarray(bq, np.float32)
    bk = np.asarray(bk, np.float32)
    bv = np.asarray(bv, np.float32)

    f8np = ml_dtypes.float8_e4m3
    xts = [np.ascontiguousarray(hs[0].T).astype(np.float16),
           np.ascontiguousarray(hs[1].T).astype(np.float16)]
    xt8s = [x.astype(f8np) for x in xts]
    tril = np.tril(np.ones((128, 128), np.float16))
    triu = np.triu(np.ones((128, 128), np.float16))
    masks = np.ascontiguousarray(np.concatenate(
        [tril, triu, tril, triu, tril, triu], axis=1))

    in_maps = []
    for core in range(N_CORES):
        b = core // 4
        h0 = HPC * (core % 4)
        wqk = np.concatenate(
            [Wq[:, h0 * 64:(h0 + 2) * 64], Wk[:, h0 * 64:(h0 + 2) * 64],
             Wq[:, (h0 + 2) * 64:(h0 + 3) * 64],
             Wk[:, (h0 + 2) * 64:(h0 + 3) * 64]],
            axis=1).astype(np.float16)
        bqk = np.zeros((3, 128), np.float32)
        bqk[0] = bq[h0 * 64:(h0 + 2) * 64]
        bqk[1] = bk[h0 * 64:(h0 + 2) * 64]
        bqk[2, 0:64] = bq[(h0 + 2) * 64:(h0 + 3) * 64]
        bqk[2, 64:128] = bk[(h0 + 2) * 64:(h0 + 3) * 64]
        wv = np.zeros((E, VW), np.float16)
        bvt1 = np.zeros((VW,), np.float16)
        for i in range(HPC):
            wv[:, 65 * i: 65 * i + 64] = Wv[:, (h0 + i) * 64:
                                            (h0 + i + 1) * 64]
            bvt1[65 * i: 65 * i + 64] = bv[(h0 + i) * 64:(h0 + i + 1) * 64]
            bvt1[65 * i + 64] = 1.0
        bvt = np.broadcast_to(np.concatenate([bvt1, bvt1]),
                              (128, 2 * VW))
        im = {
            "xt": xts[b],
            "bqk": np.ascontiguousarray(bqk),
            "wv": wv,
            "bvt": np.ascontiguousarray(bvt),
            "masks": masks,
        }
        if USE_FP8_QKPROJ:
            im["xt8"] = xt8s[b]
            im["wqk8"] = np.ascontiguousarray(wqk.astype(f8np))
        else:
            im["wqk"] = np.ascontiguousarray(wqk)
        in_maps.append(im)
    return in_maps


_NC_CACHE = None


def kernel(hidden_states, Wq, bq, Wk, bk, Wv, bv):
    global _NC_CACHE
    if _NC_CACHE is None:
        _NC_CACHE = build_program()
    nc = _NC_CACHE
    in_maps = make_in_maps(hidden_states, Wq, bq, Wk, bk, Wv, bv)
    res = None
    for attempt in range(3):
        try:
            res = bass_utils.run_bass_kernel_spmd(
                nc, in_maps, core_ids=list(range(N_CORES)))
            break
        except Exception:
            if attempt == 2:
                raise
    out = np.zeros((B, S, H * D), np.float32)
    for core in range(N_CORES):
        b = core // 4
        h0 = HPC * (core % 4)
        out[b, :, h0 * 64:(h0 + HPC) * 64] = res.results[core]["out"]
    return out
